# revision 37
# baseline (speedup 1.0000x reference)
"""Causal GQA attention (B=4, S=1024, H=16 q-heads, 4 kv-heads, D=128) on 8 trn2 cores.

Sharding: 16 (batch, kv-group) pairs -> 2 pairs/core; each pair carries 4 query
heads, so each core runs 8 independent causal-attention head-units.

Per head-unit (transposed-scores formulation, S^T[sk, sq]):
  QK^T on PE in fp8(e4m3) DoubleRow perf mode (0.5 cyc/col): q is split
    hi+lo into the two DoubleRow k-tiles (near-exact q), k single fp8 via a
    zero-stride broadcast lhsT.  Diagonal sub-blocks accumulate a second
    DoubleRow matmul with k_lo so the high-weight near-diagonal scores are
    computed to near-fp16 accuracy.
  Scores flow through [128, 512] PSUM sub-pieces (4 rotating single-bank
    buffers so the PE can run several pieces ahead of the exp engines); the
    matmul prescale makes PSUM hold 1024*log2(e)*score:
    - DVE sub-pieces: one scalar_tensor_tensor (psum + B16) * mask -> int16
      bitcast fp16 = Schraudolph exp2 with the causal mask fused.
    - ACT sub-pieces: exact exp via activation(Exp, scale=ln2/1024); for the
      sub-piece holding the diagonal block, gpsimd (Pool) zeroes the masked
      half afterwards with one 128-col fp16 multiply.
  PV on PE in fp16 with V carrying a 129th all-ones column, so the softmax
    denominator accumulates inline; the 8 (o|den) groups sit at hand-picked
    PSUM offsets that avoid bank straddles.  ACT/DVE evacuate (o|den) to
    SBUF fp16 in 3 pipelined slices and the o/den division runs on host.

The whole kernel is emitted as one flat software pipeline over (head, piece)
tasks; PV work for task g-LAG is emitted before QK+exp of task g so the
in-order PE queue always holds ready matmul work.
"""

import os
import sys

for _p in ("/opt/trn_rl_repo", "/root/.axon_site/_ro/trn_rl_repo"):
    if os.path.isdir(_p) and _p not in sys.path:
        sys.path.insert(0, _p)

from contextlib import ExitStack

import numpy as np
import ml_dtypes

import concourse.bass as bass
import concourse.tile as tile
from concourse import bacc, mybir
from concourse.bass_utils import run_bass_kernel_spmd

B = 4
S = 1024
H = 16
HKV = 4
G = H // HKV
D = 128
VW = D + 1  # V plus ones column (inline softmax denominator)
SCALE = 0.08838834764831845
LOG2E = 1.4426950408889634
NCORES = 8
PAIRS = (B * HKV) // NCORES  # 2
NU = PAIRS * G  # 8 head-units per core
NT = S // 128  # 8

# PSUM holds 1024*log2e*score; folded into host-side q/k prescale.
CSUM = 1024.0 * LOG2E * SCALE  # ~130.577
CK = 8.0
CQ = CSUM / CK
ACT_SCALE = 1.0 / (1024.0 * LOG2E)  # recovers score from psum for exact exp
# schraudolph exp2 bias (int16/fp16 domain), geometric-mean-centered so the
# DVE exp2 approximation is unbiased relative to ACT's exact exp
B16 = 15361.0 - 1024.0 * 0.05783  # ~15301.8

FP16 = mybir.dt.float16
FP32 = mybir.dt.float32
FP8 = mybir.dt.float8e4
I16 = mybir.dt.int16
DR = mybir.MatmulPerfMode.DoubleRow

# (o|den) PSUM group column offsets: 129-wide fp32 groups, chosen so no group
# crosses a 2KB PSUM bank boundary within the [128, 1312] po tile (3 banks)
GOFF = [0, 129, 258, 512, 641, 770, 1024, 1153]
# evacuation slices: (po col range, ob row range, after row-tile)
EVAC = [(0, 387, 0, 3, 2), (512, 899, 3, 6, 5), (1024, 1282, 6, 8, 7)]

# sub-pieces: (j, q-col offset within piece j, width); part 0 of each j holds
# the diagonal block in its first 128 columns
SUBS = []
for _j in range(NT):
    _total = S - 128 * _j  # diag 128 + off-diag
    _c = 0
    while _c < _total:
        _w = min(512, _total - _c)
        SUBS.append((_j, _c, _w))
        _c += _w
NSUB = len(SUBS)  # 12

PS_BUFS = 5
LAG = 5  # pv matmuls become eligible LAG steps after their dep sub-piece

_cache = {}


# evac engines (True=ACT): all DVE (ACT carries more exp instead)
EVAC_ENG = [False, False, False]


def _act_assign():
    """Balance sub-piece exps across ACT/DVE using true modeled busy costs.

    Assignment runs in piece order and picks the engine whose modeled load
    is lower, so assignments naturally alternate and neither exp engine goes
    idle for a stretch of the head.  Diagonal sub-pieces of j=0,1 are pinned
    to ACT (exact exp): they carry the entire softmax of the low-context
    rows where approximation noise shows up worst.
    """
    ta = sum((507, 528, 394)[i] for i in range(3) if EVAC_ENG[i])
    td = sum((507, 528, 394)[i] for i in range(3) if not EVAC_ENG[i])
    act = {}
    for idx in (0, 2):  # (j0, c0), (j1, c0)
        act[idx] = True
        ta += SUBS[idx][2] * 0.833 + 185
    for idx in sorted(range(NSUB), key=lambda i: -SUBS[i][2]):
        if idx in act:
            continue
        j, c, w = SUBS[idx]
        ca, cd = w * 0.833 + 185, w * 1.042 + 125
        if ta + ca <= td + cd:
            act[idx] = True
            ta += ca
        else:
            act[idx] = False
            td += cd
    return act


ACT_SUB = _act_assign()


def build_program():
    nc = bacc.Bacc("TRN2", target_bir_lowering=False, debug=False, num_devices=NCORES)

    q8_d = nc.dram_tensor("q8", [NU, 128, 2, S], FP8, kind="ExternalInput").ap()
    k8_d = nc.dram_tensor("k8", [PAIRS, 128, 2, S], FP8, kind="ExternalInput").ap()
    v16_d = nc.dram_tensor("v16", [PAIRS, 128, NT, VW], FP16, kind="ExternalInput").ap()
    mask_d = nc.dram_tensor("mask", [128, 1024], FP16, kind="ExternalInput").ap()
    o_d = nc.dram_tensor("o", [NU, 128, NT, VW], FP16, kind="ExternalOutput").ap()

    with tile.TileContext(nc) as tc, ExitStack() as ctx:
        const = ctx.enter_context(tc.tile_pool(name="const", bufs=1))
        pt_pool = ctx.enter_context(tc.tile_pool(name="pt_pool", bufs=2))
        outp = ctx.enter_context(tc.tile_pool(name="outp", bufs=2))
        ps_s = ctx.enter_context(tc.tile_pool(name="ps_s", bufs=PS_BUFS, space="PSUM"))
        ps_o = ctx.enter_context(tc.tile_pool(name="ps_o", bufs=1, space="PSUM"))

        q8s = const.tile([128, NU, 2, S], FP8)
        k8s = const.tile([128, PAIRS, 2, S], FP8)
        v16s = const.tile([128, PAIRS, NT, VW], FP16)
        # master mask [tri(128) | ones(896)]: diag sub-piece uses cols [0:512]
        mask16 = const.tile([128, 1024], FP16)

        # first loads split small so the first QK matmuls start early
        nc.sync.dma_start(out=k8s[:, 0, :, 0:256], in_=k8_d[0][:, :, 0:256])
        nc.sync.dma_start(out=q8s[:, 0, :, 0:512], in_=q8_d[0][:, :, 0:512])
        nc.sync.dma_start(out=mask16, in_=mask_d)
        nc.sync.dma_start(out=k8s[:, 0, :, 256:S], in_=k8_d[0][:, :, 256:S])
        nc.sync.dma_start(out=q8s[:, 0, :, 512:S], in_=q8_d[0][:, :, 512:S])
        nc.sync.dma_start(out=v16s[:, 0], in_=v16_d[0])
        nc.sync.dma_start(out=q8s[:, 1], in_=q8_d[1])
        nc.sync.dma_start(out=k8s[:, 1], in_=k8_d[1])
        nc.sync.dma_start(out=q8s[:, 2], in_=q8_d[2])
        nc.sync.dma_start(out=v16s[:, 1], in_=v16_d[1])
        for u in range(3, NU):
            nc.sync.dma_start(out=q8s[:, u], in_=q8_d[u])

        def k_lhsT(pair, j, lo):
            # [128, 2, 128] DoubleRow stationary: k block j broadcast over the
            # two k-tiles (matching the hi/lo q streams); lo picks k_hi/k_lo.
            blk = k8s[:, pair, lo, 128 * j : 128 * j + 128]
            return blk.rearrange("p (o k) -> p o k", o=1).broadcast_to([128, 2, 128])

        # per-head state created lazily as the flat pipeline reaches head u
        state = {}

        def get_state(u):
            if u not in state:
                state[u] = {
                    "pt": pt_pool.tile([128, NT, S], FP16, tag="pt",
                                       name=f"pt_{u}"),
                    "ob": outp.tile([128, NT, VW], FP16, tag="ob",
                                    name=f"ob_{u}"),
                    "po": ps_o.tile([128, 1312], FP32, tag="po",
                                    name=f"po_{u}"),
                }
            return state[u]

        def qk(u, j, c, w):
            # sub-piece tile: cols [c, c+w) of [diag(128) | off-diag] piece j
            pair = u // G
            qrhs = q8s[:, u]
            sp = ps_s.tile([128, 512], FP32, tag="sp", name=f"sp_{u}_{j}_{c}")
            base = 128 * j  # q-col of piece start
            cc = c
            while cc < c + w:
                ce = 128 if cc == 0 else min(cc + 256, c + w)
                # k-split: matmul 1 = (q_hi+q_lo).k_hi, matmul 2 accumulates
                # (q_hi+q_lo).k_lo -> near-fp16-accurate scores from fp8 PE
                for lo in (0, 1):
                    nc.tensor.matmul(
                        sp[:, cc - c : ce - c], lhsT=k_lhsT(pair, j, lo),
                        rhs=qrhs[:, :, base + cc : base + ce],
                        start=(lo == 0), stop=(lo == 1), perf_mode=DR,
                    )
                cc = ce
            return sp

        def exp_piece(idx, u, sp):
            j, c, w = SUBS[idx]
            pt = get_state(u)["pt"]
            q0 = 128 * j + c
            dst16 = pt[:, j, q0 : q0 + w]
            if ACT_SUB[idx]:
                nc.scalar.activation(
                    out=dst16, in_=sp[:, 0:w],
                    func=mybir.ActivationFunctionType.Exp, scale=ACT_SCALE,
                )
                if c == 0:
                    # gpsimd zeroes the masked (non-causal) half of the
                    # diagonal block before pv(j) consumes it
                    dg = pt[:, j, q0 : q0 + 128]
                    nc.gpsimd.tensor_mul(dg, dg, mask16[:, 0:128])
            else:
                # mask cols [c:c+w] of the master: tri for the diag block,
                # ones elsewhere
                nc.vector.scalar_tensor_tensor(
                    out=pt.bitcast(I16)[:, j, q0 : q0 + w],
                    in0=sp[:, 0:w], scalar=B16,
                    in1=mask16[:, c : c + w],
                    op0=mybir.AluOpType.add, op1=mybir.AluOpType.mult,
                )

        def pv(u, i):
            # atomic accumulation group: same-bank PSUM groups must not
            # interleave (interleaving corrupts the open group)
            st = get_state(u)
            pair = u // G
            for j in range(i + 1):
                nc.tensor.matmul(
                    st["po"][:, GOFF[i] : GOFF[i] + VW],
                    lhsT=st["pt"][:, j, 128 * i : 128 * i + 128],
                    rhs=v16s[:, pair, j, :],
                    start=(j == 0), stop=(j == i),
                )
            for ei, (ps, pe_, r0, r1, after) in enumerate(EVAC):
                if i == after:
                    src = st["po"][:, ps:pe_].rearrange(
                        "p (a b) -> p a b", b=VW)
                    if EVAC_ENG[ei]:
                        nc.scalar.copy(st["ob"][:, r0:r1, :], src)
                    else:
                        nc.vector.tensor_copy(st["ob"][:, r0:r1, :], src)
                    if after == EVAC[1][4]:
                        # rows 0-5 stored early; only the last 2 rows remain
                        # on the drain path after the final evac
                        nc.sync.dma_start(out=o_d[u][:, 0:6, :],
                                          in_=st["ob"][:, 0:6, :])
                    elif after == NT - 1:
                        nc.sync.dma_start(out=o_d[u][:, 6:8, :],
                                          in_=st["ob"][:, 6:8, :])
                        del state[u]

        flat = [(u, idx) for u in range(NU) for idx in range(NSUB)]

        def trailing(g):
            # pv for the diag sub-piece emitted LAG steps ago
            if 0 <= g < len(flat):
                u, idx = flat[g]
                j, c, w = SUBS[idx]
                if c == 0:
                    pv(u, j)

        for g, (u, idx) in enumerate(flat):
            trailing(g - LAG)
            j, c, w = SUBS[idx]
            exp_piece(idx, u, qk(u, j, c, w))
        for g in range(len(flat) - LAG, len(flat)):
            trailing(g)

    nc.compile()
    return nc


def _host_prep(q, k, v):
    """Shard + transpose + fp8/fp16 prescale on host; one input map per core."""
    e4 = ml_dtypes.float8_e4m3
    in_maps = []
    ii = np.arange(128)
    tri = (ii[None, :] >= ii[:, None]).astype(np.float16)  # [sk, sq]: sq >= sk
    mask = np.concatenate([tri, np.ones((128, 896), np.float16)], axis=1)
    for c in range(NCORES):
        q8 = np.empty((NU, 128, 2, S), e4)
        k8 = np.empty((PAIRS, 128, 2, S), e4)
        v16 = np.empty((PAIRS, 128, NT, VW), np.float16)
        for p in range(PAIRS):
            pg = c * PAIRS + p
            b, g = divmod(pg, HKV)
            tok = slice(b * S, (b + 1) * S)
            ks = (k[tok, g, :].astype(np.float64) * CK).astype(np.float32)
            k_hi = ks.astype(e4)
            k_lo = (ks - k_hi.astype(np.float32)).astype(e4)
            k8[p, :, 0, :] = k_hi.T
            k8[p, :, 1, :] = k_lo.T
            vseg = v[tok, g, :].astype(np.float16)  # [S, D]
            v16[p, :, :, :D] = vseg.reshape(NT, 128, D).transpose(1, 0, 2)
            v16[p, :, :, D] = np.float16(1.0)
            for hh in range(G):
                qs = (q[tok, g * G + hh, :].astype(np.float64) * CQ).astype(
                    np.float32)
                q_hi = qs.astype(e4)
                q_lo = (qs - q_hi.astype(np.float32)).astype(e4)
                u = p * G + hh
                q8[u, :, 0, :] = q_hi.T
                q8[u, :, 1, :] = q_lo.T
        in_maps.append({"q8": q8, "k8": k8, "v16": v16, "mask": mask})
    return in_maps


def _gather(results):
    out = np.empty((B * S, H, D), np.float32)
    for c in range(NCORES):
        ov = results[c]["o"].astype(np.float32)  # [NU, 128, NT, VW]
        o = ov[:, :, :, :D] / ov[:, :, :, D:D + 1]
        for p in range(PAIRS):
            pg = c * PAIRS + p
            b, g = divmod(pg, HKV)
            for hh in range(G):
                u = p * G + hh
                # o[u, sq_in_tile, i, :] -> out[b*S + 128*i + sq_in_tile]
                out[b * S : (b + 1) * S, g * G + hh, :] = (
                    o[u].transpose(1, 0, 2).reshape(S, D))
    return out


def kernel(q, k, v, cu_seqlens_q=None, cu_seqlens_k=None, **_ignored):
    if "nc" not in _cache:
        _cache["nc"] = build_program()
    nc = _cache["nc"]
    in_maps = _host_prep(np.asarray(q), np.asarray(k), np.asarray(v))
    res = run_bass_kernel_spmd(nc, in_maps, core_ids=list(range(NCORES)))
    return _gather(res.results)


# revision 44
# speedup vs baseline: 1.0047x; 1.0047x over previous
"""Causal GQA attention (B=4, S=1024, H=16 q-heads, 4 kv-heads, D=128) on 8 trn2 cores.

Sharding: 16 (batch, kv-group) pairs -> 2 pairs/core; each pair carries 4 query
heads, so each core runs 8 independent causal-attention head-units.

Per head-unit (transposed-scores formulation, S^T[sk, sq]):
  QK^T on PE in fp8(e4m3) DoubleRow perf mode (0.5 cyc/col): q is split
    hi+lo into the two DoubleRow k-tiles (near-exact q), k single fp8 via a
    zero-stride broadcast lhsT.  Diagonal sub-blocks accumulate a second
    DoubleRow matmul with k_lo so the high-weight near-diagonal scores are
    computed to near-fp16 accuracy.
  Scores flow through [128, 512] PSUM sub-pieces (5 rotating single-bank
    buffers so the PE can run several pieces ahead of the exp engines); the
    matmul prescale makes PSUM hold 1024*log2(e)*score:
    - DVE sub-pieces: one scalar_tensor_tensor (psum + B16) * mask -> int16
      bitcast fp16 = Schraudolph exp2 with the causal mask fused.
    - ACT sub-pieces: exact exp via activation(Exp, scale=ln2/1024); for the
      sub-piece holding the diagonal block, gpsimd (Pool) zeroes the masked
      half afterwards with one 128-col fp16 multiply.
  PV on PE in fp16 with V carrying a 129th all-ones column, so the softmax
    denominator accumulates inline; the 8 (o|den) groups sit at hand-picked
    PSUM offsets that avoid bank straddles.  ACT/DVE evacuate (o|den) to
    SBUF fp16 in 3 pipelined slices and the o/den division runs on host.

The whole kernel is emitted as one flat software pipeline over (head, piece)
tasks; PV work for task g-LAG is emitted before QK+exp of task g so the
in-order PE queue always holds ready matmul work.
"""

import os
import sys

for _p in ("/opt/trn_rl_repo", "/root/.axon_site/_ro/trn_rl_repo"):
    if os.path.isdir(_p) and _p not in sys.path:
        sys.path.insert(0, _p)

from contextlib import ExitStack

import numpy as np
import ml_dtypes

import concourse.bass as bass
import concourse.tile as tile
from concourse import bacc, mybir
from concourse.bass_utils import run_bass_kernel_spmd

B = 4
S = 1024
H = 16
HKV = 4
G = H // HKV
D = 128
VW = D + 1  # V plus ones column (inline softmax denominator)
SCALE = 0.08838834764831845
LOG2E = 1.4426950408889634
NCORES = 8
PAIRS = (B * HKV) // NCORES  # 2
NU = PAIRS * G  # 8 head-units per core
NT = S // 128  # 8

# PSUM holds 1024*log2e*score; folded into host-side q/k prescale.
CSUM = 1024.0 * LOG2E * SCALE  # ~130.577
CK = 8.0
CQ = CSUM / CK
ACT_SCALE = 1.0 / (1024.0 * LOG2E)  # recovers score from psum for exact exp
# schraudolph exp2 bias (int16/fp16 domain), geometric-mean-centered so the
# DVE exp2 approximation is unbiased relative to ACT's exact exp
B16 = 15361.0 - 1024.0 * 0.05783  # ~15301.8

FP16 = mybir.dt.float16
FP32 = mybir.dt.float32
FP8 = mybir.dt.float8e4
I16 = mybir.dt.int16
DR = mybir.MatmulPerfMode.DoubleRow

# (o|den) PSUM group column offsets: 129-wide fp32 groups, chosen so no group
# crosses a 2KB PSUM bank boundary within the [128, 1312] po tile (3 banks)
GOFF = [0, 129, 258, 512, 641, 770, 1024, 1153]
# evacuation slices: (po col range, ob row range, after row-tile)
EVAC = [(0, 387, 0, 3, 2), (512, 899, 3, 6, 5), (1024, 1282, 6, 8, 7)]

# sub-pieces: (j, q-col offset within piece j, width); part 0 of each j holds
# the diagonal block in its first 128 columns
SUBS = []
for _j in range(NT):
    _total = S - 128 * _j  # diag 128 + off-diag
    _c = 0
    while _c < _total:
        _w = min(512, _total - _c)
        SUBS.append((_j, _c, _w))
        _c += _w
NSUB = len(SUBS)  # 12

PS_BUFS = 5
LAG = 5  # pv matmuls become eligible LAG steps after their dep sub-piece

_cache = {}


# evac engines (True=ACT): all DVE (ACT carries more exp instead)
EVAC_ENG = [False, False, False]


def _act_assign():
    """Balance sub-piece exps across ACT/DVE using true modeled busy costs.

    Assignment runs in piece order and picks the engine whose modeled load
    is lower, so assignments naturally alternate and neither exp engine goes
    idle for a stretch of the head.  Diagonal sub-pieces of j=0,1 are pinned
    to ACT (exact exp): they carry the entire softmax of the low-context
    rows where approximation noise shows up worst.
    """
    ta = sum((507, 528, 394)[i] for i in range(3) if EVAC_ENG[i])
    td = sum((507, 528, 394)[i] for i in range(3) if not EVAC_ENG[i])
    act = {}
    for idx in (0, 2):  # (j0, c0), (j1, c0)
        act[idx] = True
        ta += SUBS[idx][2] * 0.833 + 185
    for idx in sorted(range(NSUB), key=lambda i: -SUBS[i][2]):
        if idx in act:
            continue
        j, c, w = SUBS[idx]
        ca, cd = w * 0.833 + 185, w * 1.042 + 125
        if ta + ca <= td + cd:
            act[idx] = True
            ta += ca
        else:
            act[idx] = False
            td += cd
    return act


ACT_SUB = _act_assign()


def build_program():
    nc = bacc.Bacc("TRN2", target_bir_lowering=False, debug=False, num_devices=NCORES)

    q8_d = nc.dram_tensor("q8", [NU, 128, 2, S], FP8, kind="ExternalInput").ap()
    k8_d = nc.dram_tensor("k8", [PAIRS, 128, 2, S], FP8, kind="ExternalInput").ap()
    # boot: k block 0 (hi/lo) + q8[0] cols [0:512] packed as one transfer so
    # the first QK piece starts after a single DMA latency chain
    boot_d = nc.dram_tensor("boot", [128, 2, 640], FP8, kind="ExternalInput").ap()
    v16_d = nc.dram_tensor("v16", [PAIRS, 128, NT, VW], FP16, kind="ExternalInput").ap()
    mask_d = nc.dram_tensor("mask", [128, 1024], FP16, kind="ExternalInput").ap()
    o_d = nc.dram_tensor("o", [NU, 128, NT, VW], FP16, kind="ExternalOutput").ap()

    with tile.TileContext(nc) as tc, ExitStack() as ctx:
        const = ctx.enter_context(tc.tile_pool(name="const", bufs=1))
        pt_pool = ctx.enter_context(tc.tile_pool(name="pt_pool", bufs=2))
        outp = ctx.enter_context(tc.tile_pool(name="outp", bufs=2))
        ps_s = ctx.enter_context(tc.tile_pool(name="ps_s", bufs=PS_BUFS, space="PSUM"))
        ps_o = ctx.enter_context(tc.tile_pool(name="ps_o", bufs=1, space="PSUM"))

        q8s = const.tile([128, NU, 2, S], FP8)
        k8s = const.tile([128, PAIRS, 2, S], FP8)
        v16s = const.tile([128, PAIRS, NT, VW], FP16)
        # master mask [tri(128) | ones(896)]: diag sub-piece uses cols [0:512]
        mask16 = const.tile([128, 1024], FP16)

        boot = const.tile([128, 2, 640], FP8)

        nc.sync.dma_start(out=boot, in_=boot_d)
        nc.sync.dma_start(out=q8s[:, 0, :, 512:S], in_=q8_d[0][:, :, 512:S])
        nc.sync.dma_start(out=mask16, in_=mask_d)
        nc.sync.dma_start(out=k8s[:, 0], in_=k8_d[0])
        nc.sync.dma_start(out=q8s[:, 0, :, 0:512], in_=q8_d[0][:, :, 0:512])
        nc.sync.dma_start(out=v16s[:, 0], in_=v16_d[0])
        nc.sync.dma_start(out=q8s[:, 1], in_=q8_d[1])
        nc.sync.dma_start(out=k8s[:, 1], in_=k8_d[1])
        nc.sync.dma_start(out=q8s[:, 2], in_=q8_d[2])
        nc.sync.dma_start(out=v16s[:, 1], in_=v16_d[1])
        for u in range(3, NU):
            nc.sync.dma_start(out=q8s[:, u], in_=q8_d[u])

        def k_lhsT(pair, j, lo):
            # [128, 2, 128] DoubleRow stationary: k block j broadcast over the
            # two k-tiles (matching the hi/lo q streams); lo picks k_hi/k_lo.
            blk = k8s[:, pair, lo, 128 * j : 128 * j + 128]
            return blk.rearrange("p (o k) -> p o k", o=1).broadcast_to([128, 2, 128])

        # per-head state created lazily as the flat pipeline reaches head u
        state = {}

        def get_state(u):
            if u not in state:
                state[u] = {
                    "pt": pt_pool.tile([128, NT, S], FP16, tag="pt",
                                       name=f"pt_{u}"),
                    "ob": outp.tile([128, NT, VW], FP16, tag="ob",
                                    name=f"ob_{u}"),
                    "po": ps_o.tile([128, 1312], FP32, tag="po",
                                    name=f"po_{u}"),
                }
            return state[u]

        def qk(u, j, c, w):
            # sub-piece tile: cols [c, c+w) of [diag(128) | off-diag] piece j
            pair = u // G
            qrhs = q8s[:, u]
            sp = ps_s.tile([128, 512], FP32, tag="sp", name=f"sp_{u}_{j}_{c}")
            base = 128 * j  # q-col of piece start
            first = u == 0 and j == 0  # reads from the boot tile

            def klhs(lo):
                if first:
                    blk = boot[:, lo, 0:128]
                    return blk.rearrange("p (o k) -> p o k", o=1).broadcast_to(
                        [128, 2, 128])
                return k_lhsT(pair, j, lo)

            cc = c
            while cc < c + w:
                ce = 128 if cc == 0 else min(cc + 256, c + w)
                if first and ce <= 512:
                    rhs = boot[:, :, 128 + cc : 128 + ce]
                else:
                    rhs = qrhs[:, :, base + cc : base + ce]
                # k-split: matmul 1 = (q_hi+q_lo).k_hi, matmul 2 accumulates
                # (q_hi+q_lo).k_lo -> near-fp16-accurate scores from fp8 PE
                for lo in (0, 1):
                    nc.tensor.matmul(
                        sp[:, cc - c : ce - c], lhsT=klhs(lo),
                        rhs=rhs, start=(lo == 0), stop=(lo == 1), perf_mode=DR,
                    )
                cc = ce
            return sp

        def exp_piece(idx, u, sp):
            j, c, w = SUBS[idx]
            pt = get_state(u)["pt"]
            q0 = 128 * j + c
            dst16 = pt[:, j, q0 : q0 + w]
            if ACT_SUB[idx]:
                nc.scalar.activation(
                    out=dst16, in_=sp[:, 0:w],
                    func=mybir.ActivationFunctionType.Exp, scale=ACT_SCALE,
                )
                if c == 0:
                    # gpsimd zeroes the masked (non-causal) half of the
                    # diagonal block before pv(j) consumes it
                    dg = pt[:, j, q0 : q0 + 128]
                    nc.gpsimd.tensor_mul(dg, dg, mask16[:, 0:128])
            else:
                # mask cols [c:c+w] of the master: tri for the diag block,
                # ones elsewhere
                nc.vector.scalar_tensor_tensor(
                    out=pt.bitcast(I16)[:, j, q0 : q0 + w],
                    in0=sp[:, 0:w], scalar=B16,
                    in1=mask16[:, c : c + w],
                    op0=mybir.AluOpType.add, op1=mybir.AluOpType.mult,
                )

        def pv(u, i):
            # atomic accumulation group: same-bank PSUM groups must not
            # interleave (interleaving corrupts the open group)
            st = get_state(u)
            pair = u // G
            for j in range(i + 1):
                nc.tensor.matmul(
                    st["po"][:, GOFF[i] : GOFF[i] + VW],
                    lhsT=st["pt"][:, j, 128 * i : 128 * i + 128],
                    rhs=v16s[:, pair, j, :],
                    start=(j == 0), stop=(j == i),
                )
            for ei, (ps, pe_, r0, r1, after) in enumerate(EVAC):
                if i == after:
                    src = st["po"][:, ps:pe_].rearrange(
                        "p (a b) -> p a b", b=VW)
                    if EVAC_ENG[ei]:
                        nc.scalar.copy(st["ob"][:, r0:r1, :], src)
                    else:
                        nc.vector.tensor_copy(st["ob"][:, r0:r1, :], src)
                    if after == EVAC[1][4]:
                        # rows 0-5 stored early; only the last 2 rows remain
                        # on the drain path after the final evac
                        nc.sync.dma_start(out=o_d[u][:, 0:6, :],
                                          in_=st["ob"][:, 0:6, :])
                    elif after == NT - 1:
                        nc.sync.dma_start(out=o_d[u][:, 6:8, :],
                                          in_=st["ob"][:, 6:8, :])
                        del state[u]

        flat = [(u, idx) for u in range(NU) for idx in range(NSUB)]

        def trailing(g):
            # pv for the diag sub-piece emitted LAG steps ago
            if 0 <= g < len(flat):
                u, idx = flat[g]
                j, c, w = SUBS[idx]
                if c == 0:
                    pv(u, j)

        for g, (u, idx) in enumerate(flat):
            trailing(g - LAG)
            j, c, w = SUBS[idx]
            exp_piece(idx, u, qk(u, j, c, w))
        for g in range(len(flat) - LAG, len(flat)):
            trailing(g)

    nc.compile()
    return nc


def _host_prep(q, k, v):
    """Shard + transpose + fp8/fp16 prescale on host; one input map per core."""
    e4 = ml_dtypes.float8_e4m3
    in_maps = []
    ii = np.arange(128)
    tri = (ii[None, :] >= ii[:, None]).astype(np.float16)  # [sk, sq]: sq >= sk
    mask = np.concatenate([tri, np.ones((128, 896), np.float16)], axis=1)
    for c in range(NCORES):
        q8 = np.empty((NU, 128, 2, S), e4)
        k8 = np.empty((PAIRS, 128, 2, S), e4)
        boot = np.empty((128, 2, 640), e4)
        v16 = np.empty((PAIRS, 128, NT, VW), np.float16)
        for p in range(PAIRS):
            pg = c * PAIRS + p
            b, g = divmod(pg, HKV)
            tok = slice(b * S, (b + 1) * S)
            ks = (k[tok, g, :].astype(np.float64) * CK).astype(np.float32)
            k_hi = ks.astype(e4)
            k_lo = (ks - k_hi.astype(np.float32)).astype(e4)
            k8[p, :, 0, :] = k_hi.T
            k8[p, :, 1, :] = k_lo.T
            vseg = v[tok, g, :].astype(np.float16)  # [S, D]
            v16[p, :, :, :D] = vseg.reshape(NT, 128, D).transpose(1, 0, 2)
            v16[p, :, :, D] = np.float16(1.0)
            for hh in range(G):
                qs = (q[tok, g * G + hh, :].astype(np.float64) * CQ).astype(
                    np.float32)
                q_hi = qs.astype(e4)
                q_lo = (qs - q_hi.astype(np.float32)).astype(e4)
                u = p * G + hh
                q8[u, :, 0, :] = q_hi.T
                q8[u, :, 1, :] = q_lo.T
        boot[:, :, 0:128] = k8[0, :, :, 0:128]
        boot[:, :, 128:640] = q8[0, :, :, 0:512]
        in_maps.append({"q8": q8, "k8": k8, "boot": boot, "v16": v16,
                        "mask": mask})
    return in_maps


def _gather(results):
    out = np.empty((B * S, H, D), np.float32)
    for c in range(NCORES):
        ov = results[c]["o"].astype(np.float32)  # [NU, 128, NT, VW]
        o = ov[:, :, :, :D] / ov[:, :, :, D:D + 1]
        for p in range(PAIRS):
            pg = c * PAIRS + p
            b, g = divmod(pg, HKV)
            for hh in range(G):
                u = p * G + hh
                # o[u, sq_in_tile, i, :] -> out[b*S + 128*i + sq_in_tile]
                out[b * S : (b + 1) * S, g * G + hh, :] = (
                    o[u].transpose(1, 0, 2).reshape(S, D))
    return out


def kernel(q, k, v, cu_seqlens_q=None, cu_seqlens_k=None, **_ignored):
    if "nc" not in _cache:
        _cache["nc"] = build_program()
    nc = _cache["nc"]
    in_maps = _host_prep(np.asarray(q), np.asarray(k), np.asarray(v))
    res = run_bass_kernel_spmd(nc, in_maps, core_ids=list(range(NCORES)))
    return _gather(res.results)
